# revision 2
# baseline (speedup 1.0000x reference)
"""ClusterPooling (segment-mean + gathers) on 8 TRN2 NeuronCores.

Strategy:
  - x_out = segment_mean(x, cluster_map): clusters sharded across 8 cores
    (12,500 each). Each core gathers its clusters' member rows from a
    replicated copy of x via 13 windowed `dma_gather`s (int16 indices local
    to a 32768-row window), then accumulates them into its HBM output with
    `dma_scatter_add` (CCE read-modify-write add), two duplicate-free chunks
    per window (member-rank 0 and 1 within (cluster, window)). Rare tokens of
    rank >= 2 (and any capacity overflow) are summed on the host and added to
    the downloaded sums. Division by counts is a host-side epilogue.
  - pos_out / batch_out are tiny gathers done on host; edge_index passes
    through unchanged.

The device program has a fixed, data-independent structure, so it compiles
once per process and is reused across calls.
"""

import numpy as np

# ---- problem constants (hardcoded per harness contract) ----
N = 400000
C = 256
M = 100000
N_CORES = 8
M_LOC = M // N_CORES          # 12500 clusters per core
WIN = 32768                   # int16-addressable row window
NWIN = (N + WIN - 1) // WIN   # 13
J0_COLS = 28                  # rank-0 chunk capacity: 3584 tokens
J1_COLS = 5                   # rank-1 chunk capacity: 640 tokens
WCOLS = J0_COLS + J1_COLS     # staging columns per window (33)
WSLOTS = WCOLS * 128          # 4224 slots per window
J0_SLOTS = J0_COLS * 128
J1_SLOTS = J1_COLS * 128
DUMMY0 = ((M_LOC + 127) // 128) * 128   # 12544: first dummy row
ACC_ROWS = DUMMY0 + 128                 # 12672: +128 spread dummy rows

_compiled = None  # (nc,) cached compiled program


def _build_program():
    import concourse.bass as bass
    import concourse.bacc as bacc
    import concourse.mybir as mybir
    from concourse.library_config import mlp

    nc = bacc.Bacc("TRN2", num_devices=N_CORES, num_swdge_queues=2)
    x_h = nc.dram_tensor("x", [N, C], mybir.dt.float32, kind="ExternalInput")
    gidx_h = nc.dram_tensor("gidx", [128, NWIN * WCOLS * 8], mybir.dt.int16,
                            kind="ExternalInput")
    oidx_h = nc.dram_tensor("oidx", [128, NWIN * WCOLS * 8], mybir.dt.int16,
                            kind="ExternalInput")
    acc_h = nc.dram_tensor("acc", [ACC_ROWS, C], mybir.dt.float32,
                           kind="ExternalOutput")

    with (
        nc.Block() as block,
        nc.sbuf_tensor("stg0", [128, WCOLS, C], mybir.dt.float32) as stg0,
        nc.sbuf_tensor("stg1", [128, WCOLS, C], mybir.dt.float32) as stg1,
        nc.sbuf_tensor("gidx_s", [128, NWIN * WCOLS * 8], mybir.dt.int16) as gidx_s,
        nc.sbuf_tensor("oidx_s", [128, NWIN * WCOLS * 8], mybir.dt.int16) as oidx_s,
        nc.semaphore("io") as io,
        nc.semaphore("gsem") as gsem,
        nc.semaphore("ssem") as ssem,
    ):
        stgs = [stg0, stg1]

        @block.gpsimd
        def _(gpsimd: bass.BassGpSimd):
            gpsimd.load_library(mlp)
            gpsimd.dma_start(gidx_s[:], gidx_h[:]).then_inc(io, 16)
            gpsimd.dma_start(oidx_s[:], oidx_h[:]).then_inc(io, 16)
            gpsimd.wait_ge(io, 32)

            def issue_gather(w):
                lo = w * WIN
                hi = min(lo + WIN, N)
                gpsimd.dma_gather(
                    stgs[w % 2][:, :WCOLS, :],
                    x_h.ap()[lo:hi, :],
                    gidx_s[:, w * WCOLS * 8:(w + 1) * WCOLS * 8],
                    WSLOTS, WSLOTS, C,
                    single_packet=False,
                    queue_num=0,
                ).then_inc(gsem, 16)

            issue_gather(0)
            for w in range(NWIN):
                # gather w drained
                gpsimd.wait_ge(gsem, 16 * (w + 1))
                # chunks of window w-1 drained (frees stg[(w+1)%2], orders chain)
                if w >= 1:
                    gpsimd.wait_ge(ssem, 16 * 2 * w)
                if w + 1 < NWIN:
                    issue_gather(w + 1)
                ob = w * WCOLS * 8
                gpsimd.dma_scatter_add(
                    acc_h.ap(),
                    stgs[w % 2][:, :J0_COLS, :],
                    oidx_s[:, ob:ob + J0_COLS * 8],
                    J0_SLOTS, J0_SLOTS, C,
                    single_packet=False,
                    queue_num=1,
                ).then_inc(ssem, 16)
                gpsimd.wait_ge(ssem, 16 * (2 * w + 1))
                gpsimd.dma_scatter_add(
                    acc_h.ap(),
                    stgs[w % 2][:, J0_COLS:WCOLS, :],
                    oidx_s[:, ob + J0_COLS * 8:ob + WCOLS * 8],
                    J1_SLOTS, J1_SLOTS, C,
                    single_packet=False,
                    queue_num=1,
                ).then_inc(ssem, 16)
            gpsimd.wait_ge(ssem, 16 * 2 * NWIN)

    nc.compile()
    return nc


def _wrap16(vals_by_slot, ncols):
    """Slot-space -> idx-wrap layout. slot k: p=k%128, col=k//128;
    wrap[r, 8*col + q] with q=p//16, r=p%16; tiled to 128 partitions."""
    n = ncols * 128
    w = np.zeros((16, n // 16), dtype=np.int16)
    k = np.arange(n)
    p = k % 128
    col = k // 128
    w[p % 16, 8 * col + p // 16] = vals_by_slot
    return np.tile(w, (8, 1))


def _build_core_plan(rows, cl):
    """rows: absolute x-row ids of this core's member tokens; cl: local cluster
    ids. Returns (gidx [128, NWIN*WCOLS*8] i16, oidx same, host_leftover_mask)."""
    w = rows // WIN
    # rank of each token within (window, cluster)
    order = np.lexsort((cl, w))
    ws, cs, rs = w[order], cl[order], rows[order]
    key = ws.astype(np.int64) * M_LOC + cs
    new = np.ones(len(key), bool)
    if len(key) > 1:
        new[1:] = key[1:] != key[:-1]
    run_start = np.maximum.accumulate(np.where(new, np.arange(len(key)), 0))
    rank = np.arange(len(key)) - run_start

    gslots = np.zeros(NWIN * WSLOTS, dtype=np.int16)       # pad: row 0
    oslots = np.empty(NWIN * WSLOTS, dtype=np.int16)
    oslots[:] = (DUMMY0 + np.arange(NWIN * WSLOTS) % 128).astype(np.int16)
    leftover = np.zeros(len(key), bool)

    for wi in range(NWIN):
        for j, (base, cap) in ((0, (0, J0_SLOTS)), (1, (J0_SLOTS, J1_SLOTS))):
            sel = (ws == wi) & (rank == j)
            n = int(sel.sum())
            if n == 0:
                continue
            idxs = np.flatnonzero(sel)
            if n > cap:
                leftover[idxs[cap:]] = True
                idxs = idxs[:cap]
                n = cap
            s0 = wi * WSLOTS + base
            gslots[s0:s0 + n] = (rs[idxs] - wi * WIN).astype(np.int16)
            oslots[s0:s0 + n] = cs[idxs].astype(np.int16)
    leftover |= rank >= 2

    gidx = np.concatenate(
        [_wrap16(gslots[wi * WSLOTS:(wi + 1) * WSLOTS], WCOLS) for wi in range(NWIN)],
        axis=1)
    # scatter wraps are per-chunk (local col index within chunk)
    oparts = []
    for wi in range(NWIN):
        s0 = wi * WSLOTS
        oparts.append(_wrap16(oslots[s0:s0 + J0_SLOTS], J0_COLS))
        oparts.append(_wrap16(oslots[s0 + J0_SLOTS:s0 + WSLOTS], J1_COLS))
    oidx = np.concatenate(oparts, axis=1)
    # leftover tokens in original (unsorted) index space
    left_rows = rs[leftover]
    left_cl = cs[leftover]
    return gidx, oidx, left_rows, left_cl


def kernel(x, pos, batch, cluster_map, sample_index, edge_index, num_clusters):
    global _compiled
    from concourse.bass_utils import run_bass_kernel_spmd

    x = np.ascontiguousarray(np.asarray(x, dtype=np.float32))
    pos = np.asarray(pos)
    batch = np.asarray(batch)
    cm = np.asarray(cluster_map)
    si = np.asarray(sample_index)

    if _compiled is None:
        _compiled = _build_program()
    nc = _compiled

    cm64 = cm.astype(np.int64)
    core_of = cm64 // M_LOC
    cl_loc = cm64 % M_LOC
    all_rows = np.arange(N, dtype=np.int64)

    in_maps = []
    leftovers = []
    for i in range(N_CORES):
        sel = core_of == i
        gidx, oidx, lrows, lcl = _build_core_plan(all_rows[sel], cl_loc[sel])
        in_maps.append({"x": x, "gidx": gidx, "oidx": oidx})
        leftovers.append((lrows, lcl + i * M_LOC))

    res = run_bass_kernel_spmd(nc, in_maps, list(range(N_CORES)))

    sums = np.concatenate(
        [res.results[i]["acc"][:M_LOC] for i in range(N_CORES)], axis=0)
    # host-side leftovers (rank>=2 within (cluster, window) + overflow)
    for lrows, lclg in leftovers:
        if len(lrows):
            np.add.at(sums, lclg, x[lrows])

    counts = np.bincount(cm64, minlength=M).astype(np.float32)
    x_out = sums / np.maximum(counts, np.float32(1.0))[:, None]

    pos_out = pos[si]
    batch_out = batch[si]
    return x_out.astype(np.float32), pos_out, batch_out, edge_index
